# revision 2
# baseline (speedup 1.0000x reference)
"""Involution-style aggregation (nn_AggregationNonCupy) on 8 Trainium2 NeuronCores.

out[n, g*32+cw, y, x] = sum_{di,dj in {-1,0,1}} weight[n, cw, (di+1)*3+dj+1, y*64+x]
                        * input[n, g*32+cw, y+di, x+dj]      (zero padded)

v3 design — DMA-optimized layout (HW evidence: DMA is limited ~100 GB/s/core
for 3-dim-AP transfers regardless of descriptor size, but uniform-stride
2-dim-AP transfers reach ~330-370 GB/s):
  - Sharding: core = (batch-quad bq, group-half gh): batches 4bq..4bq+3,
    channels 256gh..256gh+255 (8 groups of 32). The weight slice [4 batches]
    is read by both gh cores (2x weight HBM reads, buys the layout below).
  - Partition dim = (n4, cw) = 128. Per group, each partition processes the
    FULL 64x64 channel map in the free dim. All DRAM transfers are 2-dim
    uniform-stride APs: input/output one instr per (group, n) = [32 x 16KB],
    weights one instr per 3 taps = [128 x 49KB fp32] SWDGE casting DMAs
    (fp32->fp16 in the DMA engine, verified bit-exact round-to-nearest).
  - Input SBUF block per group: [pad row, 64 rows, pad row, 2] = 4226 fp16
    elems. Vertical taps are free-dim offsets (no halo duplication or
    re-reads); horizontal x-boundary taps use column-zeroed weights; odd-dj
    alignment for DVE 2x mode uses a +1-shifted fp16 copy (DVE tensor_copy
    runs at 4x for packed fp16: ~1.26us per block).
  - Products fp16: DVE 2x (14 chunks/group) + Pool (4 chunks/group);
    tap accumulation on PE via identity matmul into PSUM fp32; ACT converts
    input fp32->fp16 and evacuates PSUM.
"""

import os

import numpy as np

import concourse.bacc as bacc
import concourse.mybir as mybir
import concourse.tile as tile
from concourse.bass_utils import run_bass_kernel_spmd

# Problem constants (hardcoded per harness contract)
N_TOTAL, C_X, H, W = 16, 512, 64, 64
C_W = 32
N_CORES = 8
N4 = 4             # batches per core
G8 = 8             # groups per core
HW_ = H * W        # 4096
IM = 2 + W         # image offset in the block (4B-aligned for the cast DMA)
BLK = IM + HW_ + W + 2  # 4228: [z2, pad row, image, pad row, z2]
WCOLS = 9 * HW_    # 36864 weight elems per partition
MM_N = 512         # matmul chunk (one PSUM bank of fp32)
HALF = HW_ // 2    # 2048

# tap list: k = (di+1)*3 + (dj+1)
TAPS = [(di, dj) for di in (-1, 0, 1) for dj in (-1, 0, 1)]

# All products run on DVE: Pool's tensor_mul measured ~10us per [128,2048]
# chunk on HW (vs DVE 1.13us at 2x), and any Pool share serialized the flow.
POOL_BY_GROUP = [set() for _ in range(8)]
PIPE = 2           # input-stage lookahead (groups)
# debug flag set (comma-separated): nodma, nocompute, nomm, noevac, alldve
MODE = {"full"}


def emit_kernel(tc, x, wgt, o, reps=1):
    nc = tc.nc
    f32 = mybir.dt.float32
    f16 = mybir.dt.float16

    # x/o arrive host-permuted as [g, (n cw), l]: per-group transfers are a
    # single fully-contiguous 2MB uniform-stride instruction (~366 GB/s HW
    # path; 3-dim APs only reach ~100 GB/s)
    xv = x
    ov = o
    wv = wgt.rearrange("n cw k l -> (n cw) (k l)")        # [128, 36864]

    ident_dram = nc.inline_tensor(np.eye(128, dtype=np.float16), name="ident")

    with (
        tc.tile_pool(name="const", bufs=1) as const_pool,
        tc.tile_pool(name="w16", bufs=1) as w16_pool,
        tc.tile_pool(name="ina", bufs=4) as ina_pool,
        tc.tile_pool(name="prod", bufs=6) as prod_pool,
        tc.tile_pool(name="psum", bufs=2, space="PSUM") as psum_pool,
        tc.tile_pool(name="wst", bufs=2) as wst_pool,
        tc.tile_pool(name="outp", bufs=2) as out_pool,
    ):
        ident = const_pool.tile([128, 128], f16)
        nc.sync.dma_start(ident[:], ident_dram.ap())

        env = dict(locals())

        if reps == 1:
            _emit_body(tc, env)
        else:
            with tc.For_i(0, reps, 1):
                _emit_body(tc, env)


def _emit_body(tc, env):
    nc = env["nc"]
    f32, f16 = env["f32"], env["f16"]
    xv, ov, wv, ident = env["xv"], env["ov"], env["wv"], env["ident"]
    w16_pool = env["w16_pool"]
    ina_pool = env["ina_pool"]
    prod_pool, psum_pool, out_pool = (env["prod_pool"], env["psum_pool"],
                                      env["out_pool"])
    wst_pool = env["wst_pool"]

    # ---- weights: fp16 resident tile [128, 36864], SWDGE casting loads
    wt16 = w16_pool.tile([128, WCOLS], f16, tag="wt16")

    def load_weights():
        # fp32 loads on the SP HWDGE queue (idle at body start; the SWDGE
        # queue is reserved for input casts) + ACT fp32->fp16 conversion.
        # Per-tap chunks so the first products unblock after one chunk.
        wview = wt16.rearrange("p (k y xx) -> p k y xx", k=9, xx=W)
        for k, (di, dj) in enumerate(TAPS):
            if "nodma" not in MODE:
                ws = wst_pool.tile([128, HW_], f32, tag="ws")
                nc.sync.dma_start(ws[:], wv[:, k * HW_:(k + 1) * HW_])
                if "nocompute" in MODE:
                    continue
                nc.scalar.copy(wt16[:, k * HW_:(k + 1) * HW_], ws[:])
            if "nocompute" in MODE:
                continue
            # zero weight columns at x-boundaries: dj=-1 taps kill x=0,
            # dj=+1 taps kill x=63 (their input reads are out-of-range wraps)
            if dj != 0:
                col = 0 if dj == -1 else W - 1
                nc.gpsimd.memset(wview[:, k, :, col:col + 1], 0.0)

    stage = {}

    def input_stage(g):
        ita = ina_pool.tile([128, BLK], f16, tag="ita")
        nc.gpsimd.memset(ita[:, 0:IM], 0.0)
        nc.gpsimd.memset(ita[:, IM + HW_:BLK], 0.0)
        # fp32->fp16 casting DMA (SWDGE), fully-contiguous 2MB source
        if "nodma" not in MODE:
            nc.gpsimd.dma_start(ita[:, IM:IM + HW_], xv[g])
        stage[g] = ita

    def compute_stage(g):
        ita = stage.pop(g)
        pool_chunks = POOL_BY_GROUP[g]
        ot = out_pool.tile([128, HW_], f32, tag="ot")
        if "nocompute" in MODE:
            if "nodma" not in MODE:
                nc.sync.dma_start(ov[g], ot[:])
            return
        use_pool = "alldve" not in MODE

        def src_for(k, half):
            # odd-elem offsets verified to keep DVE 2x mode on HW, so the
            # dj=+-1 taps read ita directly (no shifted copy needed).
            # Edge reads land in the zeroed pad rows / lead+tail elems.
            di, dj = TAPS[k]
            s = IM + W * di + dj + half * HALF
            return ita[:, s:s + HALF]

        def wk_for(k, half):
            return wt16[:, k * HW_ + half * HALF:k * HW_ + (half + 1) * HALF]

        pool_pk = {}

        for half in range(2):
            ps = psum_pool.tile([128, HALF], f32, tag="ps")
            # DVE taps first (available soonest), Pool taps consumed last —
            # the tap sum is order-independent, so this gives Pool products
            # maximum slack before PE needs them
            order = [(k, half) for k in range(9)
                     if not (use_pool and (k, half) in pool_chunks)]
            order += sorted(kh for kh in pool_chunks
                            if kh[1] == half and use_pool)
            for i, (k, half_) in enumerate(order):
                if (k, half_) in pool_pk:
                    pk = pool_pk[(k, half_)]
                else:
                    pk = prod_pool.tile([128, HALF], f16, tag="pk")
                    nc.vector.tensor_mul(pk[:], wk_for(k, half_),
                                         src_for(k, half_))
                if "nomm" in MODE:
                    continue
                for c in range(0, HALF, MM_N):
                    nc.tensor.matmul(ps[:, c:c + MM_N], ident[:],
                                     pk[:, c:c + MM_N],
                                     start=(i == 0), stop=(i == len(order) - 1))
            if "nomm" in MODE or "noevac" in MODE:
                continue
            nc.scalar.copy(ot[:, half * HALF:(half + 1) * HALF], ps[:])
        # fully-contiguous 2MB store (host un-permutes); SP HWDGE queue is
        # otherwise idle (input+weights ride the Pool SWDGE queue)
        if not MODE & {"nodma", "nomm", "noevac"}:
            nc.sync.dma_start(ov[g], ot[:])

    # startup: interleave the weight-cast chunks with the first input loads
    # on the shared SWDGE queue so the first products start after
    # in0 + w[taps 0-2] rather than after the whole weight phase
    input_stage(0)
    load_weights()
    for g in range(1, min(PIPE, G8)):
        input_stage(g)
    for g in range(G8):
        if g + PIPE < G8:
            input_stage(g + PIPE)
        compute_stage(g)


def build_program(reps=1):
    nc = bacc.Bacc("TRN2", target_bir_lowering=False, debug=False,
                   enable_asserts=True, num_devices=N_CORES)
    f32 = mybir.dt.float32
    x = nc.dram_tensor("x", [G8, 128, HW_], f32, kind="ExternalInput").ap()
    wgt = nc.dram_tensor("w", [N4, C_W, 9, HW_], f32,
                         kind="ExternalInput").ap()
    o = nc.dram_tensor("o", [G8, 128, HW_], f32, kind="ExternalOutput").ap()
    with tile.TileContext(nc) as tc:
        emit_kernel(tc, x, wgt, o, reps=reps)
    nc.compile()
    return nc


_CACHED_NC = None


def _get_nc():
    global _CACHED_NC
    if _CACHED_NC is None:
        _CACHED_NC = build_program()
    return _CACHED_NC


def run(inputs, trace=False):
    """Run on 8 cores; returns (output [16,512,64,64] fp32, BassKernelResults)."""
    inp = np.ascontiguousarray(np.asarray(inputs["input"], dtype=np.float32))
    wgt = np.ascontiguousarray(np.asarray(inputs["weight"], dtype=np.float32))
    assert inp.shape == (N_TOTAL, C_X, H, W)
    assert wgt.shape == (N_TOTAL, C_W, 9, HW_)

    nc = _get_nc()
    inp = inp.reshape(N_TOTAL, C_X, HW_)
    in_maps = []
    for c in range(N_CORES):
        bq, gh = divmod(c, 2)
        nsl = slice(4 * bq, 4 * bq + 4)
        csl = slice(256 * gh, 256 * gh + 256)
        # host-permute input to [g, n, cw, l] so device transfers are
        # fully contiguous per group
        xc = inp[nsl, csl].reshape(N4, G8, C_W, HW_).transpose(1, 0, 2, 3)
        in_maps.append({
            "x": np.ascontiguousarray(xc).reshape(G8, 128, HW_),
            "w": np.ascontiguousarray(wgt[nsl]),
        })
    res = run_bass_kernel_spmd(nc, in_maps, core_ids=list(range(N_CORES)),
                               trace=trace)
    out = np.empty((N_TOTAL, C_X, HW_), dtype=np.float32)
    for c in range(N_CORES):
        bq, gh = divmod(c, 2)
        oc = res.results[c]["o"].reshape(G8, N4, C_W, HW_).transpose(1, 0, 2, 3)
        out[4 * bq:4 * bq + 4, 256 * gh:256 * gh + 256] = \
            oc.reshape(N4, 256, HW_)
    return out.reshape(N_TOTAL, C_X, H, W), res


def kernel(**inputs):
    out, _ = run(inputs)
    return out
